# revision 16
# baseline (speedup 1.0000x reference)
"""Trainium2 Bass kernel for the 2-layer heterogeneous GNN (GATv2 + CGConv).

Sharding: destination nodes (both node types) are split into 8 contiguous
ranges of 2560 (N padded 20000 -> 20480); each core owns the edges that
target its range, for all 4 relations.  Node features are replicated
(SBUF-resident, bf16, node-wrapped layout); the one inter-layer halo
exchange is a single AllGather of the updated 2560-row slices.

Within a core, dst nodes are PERMUTED into 20 tiles of 128 so that each
tile's incoming edge count (per relation) is balanced -- most tiles then
need only 4 edge-blocks of 128 (vs 5 unbalanced).  One-hot edge->dst
selectors are built on the host and streamed from DRAM in two orientations:
  oh_agg [e_p, d_f]  - lhsT of the aggregation matmul (segment sum)
  oh_sel [d_p, e_f]  - lhsT of the dst-feature-select matmul, which
                       replaces the dst-side per-edge gather entirely:
                       psz[e,:] += oh_sel^T @ (x_dst_tile @ W_dst).
Only the src side is gathered (gpsimd dma_gather, feature-major).

GATv2 aggregates the alpha-weighted SUM tensor psz = xl + xr directly and
recovers sum(alpha*xl) as (sum(es*psz) - xrW*s) / s per dst (softmax
weights sum to s per dst), which removes the per-block mid-PSUM copy of
xl.  Per-edge elementwise work is batched per 2-tile chunk (broadcast APs
for the alpha scaling, strided views for the CGConv gate/softplus halves),
with activation functions grouped to avoid ACT table reloads.
"""

import os
import numpy as np
import ml_dtypes

BF = ml_dtypes.bfloat16

N = 20000
D = 128
H = 4
L = 2
E = 80000
CORES = 8
NPAD = 20480
SHARD = 2560
TILES = 20           # dst tiles of 128 per core
TPG = 4              # tiles per gather group
TPC = 2              # tiles per compute chunk
NGR = TILES // TPG   # gather groups per relation (5)
RANKS = NPAD // 128  # 160
PAD_NODE = 20000     # zero-feature padding node (valid gather target)

LAST_EXEC_NS = None

# relation table: (name, kind, src_type, dst_type); CG first per dst type
RELS = [
    ("loses", "cg", "my", "opp"),
    ("beats", "gat", "my", "opp"),
    ("rev_beats", "cg", "opp", "my"),
    ("rev_loses", "gat", "opp", "my"),
]


# ----------------------------------------------------------------- host prep

def _balance(c1, c2):
    """Assign 2560 local nodes to 20 tiles of 128, balancing both relations'
    per-tile edge counts.  Tiles are then sorted heaviest-first so that the
    per-tile-index block counts align across cores (BPT is maxed over
    cores).  Returns assign[2560] -> tile id."""
    order = np.argsort(-np.maximum(c1, c2), kind="stable")
    l1 = np.zeros(TILES, np.int64)
    l2 = np.zeros(TILES, np.int64)
    cnt = np.zeros(TILES, np.int64)
    assign = np.empty(SHARD, np.int64)
    # skewed capacity profile: surplus beyond 20*512 concentrates in the
    # leading tiles, which later align across cores (heaviest-first order)
    capv = np.full(TILES, 4 * 128, np.float64)
    capv[0], capv[1], capv[2] = 6 * 128, 5 * 128, 5 * 128
    for n in order:
        m1, m2 = l1 + c1[n], l2 + c2[n]
        f = np.maximum(np.maximum(m1 / capv, m2 / capv), (cnt + 1) / 128.0)
        ok = (m1 <= capv) & (m2 <= capv) & (cnt < 128)
        if ok.any():
            f = np.where(ok, f, np.inf)
        else:
            f = np.maximum(m1, m2).astype(np.float64)
            f[cnt >= 128] = np.inf
        t = int(np.argmin(f))
        assign[n] = t
        l1[t] += c1[n]
        l2[t] += c2[n]
        cnt[t] += 1
    # heaviest-first tile ordering (aligns block counts across cores)
    n1 = -(-l1 // 128)
    n2 = -(-l2 // 128)
    key = (n1 + n2) * 10000 + np.maximum(n1, n2) * 100
    rank = np.empty(TILES, np.int64)
    rank[np.argsort(-key, kind="stable")] = np.arange(TILES)
    return rank[assign]


def _wrap_nodes(xg):
    """position-ordered features [NPAD, D] f32 -> node-wrapped
    [128, RANKS*D] bf16 (position p at partition p%128, cols (p//128)*D)."""
    return np.ascontiguousarray(
        xg.reshape(RANKS, 128, D).transpose(1, 0, 2).reshape(128, RANKS * D)
    ).astype(BF)


def _idx_dev(a):
    """[EP] int -> [128, EP//16] int16 (16-partition wrap, replicated 8x)."""
    x = a.astype(np.int16).reshape(-1, 16).T
    return np.ascontiguousarray(np.tile(x, (8, 1)))


def _rep(v, rows=128):
    return np.ascontiguousarray(
        np.tile(np.asarray(v, np.float32).reshape(1, -1), (rows, 1)))


def _prep_graph(inputs):
    """Permutations, per-core packed edges, one-hots, BPT config."""
    ei = {r: np.asarray(inputs[k]).astype(np.int64)
          for r, k in (("loses", "ei_loses"), ("beats", "ei_beats"),
                       ("rev_beats", "ei_rev_beats"),
                       ("rev_loses", "ei_rev_loses"))}
    rels_of_type = {"opp": ("beats", "loses"), "my": ("rev_beats", "rev_loses")}

    assign = {}      # (ty, c) -> [SHARD] tile id
    pos_local = {}   # (ty, c) -> [SHARD] permuted position within shard
    order_ids = {}   # (ty, c) -> [SHARD] local node id at each position
    for ty, (r1, r2) in rels_of_type.items():
        d1 = np.bincount(ei[r1][1], minlength=NPAD)
        d2 = np.bincount(ei[r2][1], minlength=NPAD)
        for c in range(CORES):
            sl = slice(c * SHARD, (c + 1) * SHARD)
            a = _balance(d1[sl], d2[sl])
            assign[(ty, c)] = a
            order = np.argsort(a * SHARD + np.arange(SHARD), kind="stable")
            order_ids[(ty, c)] = order
            p = np.empty(SHARD, np.int64)
            p[order] = np.arange(SHARD)
            pos_local[(ty, c)] = p

    pos_g = {}
    for ty in ("my", "opp"):
        pg = np.empty(NPAD, np.int64)
        for c in range(CORES):
            pg[c * SHARD:(c + 1) * SHARD] = c * SHARD + pos_local[(ty, c)]
        pos_g[ty] = pg

    # per-tile block counts, maxed over cores (program must be SPMD-uniform)
    BPT = {r: np.zeros(TILES, np.int64) for r in ei}
    edges = {}
    for rname, kind, sty, dty in RELS:
        src, dst = ei[rname]
        for c in range(CORES):
            m = (dst >= c * SHARD) & (dst < (c + 1) * SHARD)
            s, d = src[m], dst[m] - c * SHARD
            t = assign[(dty, c)][d]
            r = pos_local[(dty, c)][d] % 128
            edges[(rname, c)] = (s, t, r)
            cnt = np.bincount(t, minlength=TILES)
            BPT[rname] = np.maximum(BPT[rname], -(-cnt // 128))
        BPT[rname] = np.maximum(BPT[rname], 1)

    packed = {}
    for rname, kind, sty, dty in RELS:
        bpt = BPT[rname]
        boff = np.concatenate([[0], np.cumsum(bpt)])
        EP = int(boff[-1]) * 128
        for c in range(CORES):
            s, t, r = edges[(rname, c)]
            o = np.argsort(t, kind="stable")
            s, t, r = s[o], t[o], r[o]
            cnts = np.bincount(t, minlength=TILES)
            idx_in_tile = np.concatenate(
                [np.arange(cnts[tt]) for tt in range(TILES)])
            slot = boff[t] * 128 + idx_in_tile
            si_pos = np.full(EP, pos_g[sty][PAD_NODE], np.int64)
            si_pos[slot] = pos_g[sty][s]
            ohf = np.zeros((EP, 128), np.float32)
            ohf[slot, r] = 1.0
            ohb = ohf.reshape(-1, 128, 128)
            oh_agg = np.ascontiguousarray(
                ohb.transpose(1, 0, 2).reshape(128, EP)).astype(BF)
            oh_sel = np.ascontiguousarray(
                ohb.transpose(2, 0, 1).reshape(128, EP)).astype(BF)
            packed[(rname, c)] = (_idx_dev(si_pos), oh_agg, oh_sel)

    return BPT, packed, pos_g, order_ids


# ------------------------------------------------------------- program build

def _build_program(cfg):
    import concourse.bass as bass
    import concourse.bacc as bacc
    import concourse.mybir as mybir
    import concourse.tile as tile

    F32, BF16, I16 = mybir.dt.float32, mybir.dt.bfloat16, mybir.dt.int16
    AF = mybir.ActivationFunctionType
    OP = mybir.AluOpType

    BPT = {r[0]: list(v) for r, v in zip(RELS, cfg)}
    boff = {}
    for r in BPT:
        boff[r] = [0]
        for t in range(TILES):
            boff[r].append(boff[r][t] + BPT[r][t])
    EP = {r: boff[r][-1] * 128 for r in BPT}
    # max blocks per compute chunk / gather group
    NBC = max(boff[r][t + TPC] - boff[r][t]
              for r in BPT for t in range(0, TILES, TPC))
    NBG = max(boff[r][t + TPG] - boff[r][t]
              for r in BPT for t in range(0, TILES, TPG))

    k_layers = int(os.environ.get("K_LAYERS", str(L)))
    k_rels = os.environ.get("K_RELS", "")
    rels_active = [r for r in RELS if (not k_rels or r[0] in k_rels.split(","))]

    nc = bacc.Bacc("TRN2", target_bir_lowering=False, debug=False,
                   num_devices=CORES)

    dr = {}
    for ty in ("my", "opp"):
        dr[f"xw_{ty}"] = nc.dram_tensor(f"xw_{ty}", [128, RANKS * D], BF16,
                                        kind="ExternalInput")
        dr[f"xres_{ty}"] = nc.dram_tensor(f"xres_{ty}", [128, TILES * D], BF16,
                                          kind="ExternalInput")
        dr[f"xfm_{ty}"] = nc.dram_tensor(f"xfm_{ty}", [128, TILES * D], BF16,
                                         kind="ExternalInput")
    for rname, kind, _, _ in RELS:
        dr[f"si_{rname}"] = nc.dram_tensor(
            f"si_{rname}", [128, EP[rname] // 16], I16, kind="ExternalInput")
        dr[f"ohA_{rname}"] = nc.dram_tensor(
            f"ohA_{rname}", [128, EP[rname]], BF16, kind="ExternalInput")
        dr[f"ohS_{rname}"] = nc.dram_tensor(
            f"ohS_{rname}", [128, EP[rname]], BF16, kind="ExternalInput")
        if kind == "gat":
            dr[f"wl_{rname}"] = nc.dram_tensor(f"wl_{rname}", [L, 128, H * D], BF16, kind="ExternalInput")
            dr[f"wr_{rname}"] = nc.dram_tensor(f"wr_{rname}", [L, 128, H * D], BF16, kind="ExternalInput")
            dr[f"att_{rname}"] = nc.dram_tensor(f"att_{rname}", [L, 128, H * D], BF16, kind="ExternalInput")
            dr[f"gb_{rname}"] = nc.dram_tensor(f"gb_{rname}", [L, 128, D], F32, kind="ExternalInput")
        else:
            dr[f"wt_{rname}"] = nc.dram_tensor(f"wt_{rname}", [L, 128, 2 * D], BF16, kind="ExternalInput")
            dr[f"wb_{rname}"] = nc.dram_tensor(f"wb_{rname}", [L, 128, 2 * D], BF16, kind="ExternalInput")
            dr[f"cb_{rname}"] = nc.dram_tensor(f"cb_{rname}", [L, 1, 2 * D], BF16, kind="ExternalInput")
    dr["nw_w"] = nc.dram_tensor("nw_w", [L, 128, D], BF16, kind="ExternalInput")
    dr["nw_b"] = nc.dram_tensor("nw_b", [L, 128, 1], F32, kind="ExternalInput")
    dr["ident_f"] = nc.dram_tensor("ident_f", [128, 128], F32, kind="ExternalInput")
    dr["ident_b"] = nc.dram_tensor("ident_b", [128, 128], BF16, kind="ExternalInput")
    dr["out_my"] = nc.dram_tensor("out_my", [SHARD, D], F32, kind="ExternalOutput")
    dr["out_opp"] = nc.dram_tensor("out_opp", [SHARD, D], F32, kind="ExternalOutput")

    def ld3(pool, name, src, cols):
        t = pool.tile([128, L * cols], src.dtype, name=name, tag=name)
        nc.sync.dma_start(
            t[:].rearrange("p (l n) -> p l n", l=L),
            src[:].rearrange("l p n -> p l n"),
        )
        return t

    from contextlib import ExitStack

    with tile.TileContext(nc) as tc:
        with ExitStack() as _st:
            cst = _st.enter_context(tc.tile_pool(name="const", bufs=1))
            xwp = _st.enter_context(tc.tile_pool(name="xwp", bufs=1))
            accp = _st.enter_context(tc.tile_pool(name="accp", bufs=1))
            gth = _st.enter_context(tc.tile_pool(name="gth", bufs=2))
            sip = _st.enter_context(tc.tile_pool(name="sip", bufs=2))
            ohp = _st.enter_context(tc.tile_pool(name="ohp", bufs=2))
            xdp = _st.enter_context(tc.tile_pool(name="xdp", bufs=2))
            prp = _st.enter_context(tc.tile_pool(name="prp", bufs=2))
            zp = _st.enter_context(tc.tile_pool(name="zp", bufs=1))
            wrk = _st.enter_context(tc.tile_pool(name="wrk", bufs=2))
            cgs = _st.enter_context(tc.tile_pool(name="cgs", bufs=1))
            epi = _st.enter_context(tc.tile_pool(name="epi", bufs=2))
            drm = _st.enter_context(tc.tile_pool(name="dram", bufs=1, space="DRAM"))
            pzp = _st.enter_context(tc.tile_pool(name="pz", bufs=2, space=bass.MemorySpace.PSUM))
            paggp = _st.enter_context(tc.tile_pool(name="pagg", bufs=2, space=bass.MemorySpace.PSUM))
            pssp = _st.enter_context(tc.tile_pool(name="pss", bufs=2, space=bass.MemorySpace.PSUM))
            psp = _st.enter_context(tc.tile_pool(name="ps", bufs=2, space=bass.MemorySpace.PSUM))

            # ---------------- constants / inputs resident in SBUF
            xw, xres, xfm = {}, {}, {}
            for ty in ("my", "opp"):
                xw[ty] = xwp.tile([128, RANKS * D], BF16, name=f"xw_{ty}_sb", tag=f"xw_{ty}_sb")
                nc.sync.dma_start(xw[ty][:], dr[f"xw_{ty}"][:])
                xres[ty] = xwp.tile([128, TILES * D], BF16, name=f"xres_{ty}_sb", tag=f"xres_{ty}_sb")
                nc.sync.dma_start(xres[ty][:], dr[f"xres_{ty}"][:])
                xfm[ty] = xwp.tile([128, TILES * D], BF16, name=f"xfm_{ty}_sb", tag=f"xfm_{ty}_sb")
                nc.sync.dma_start(xfm[ty][:], dr[f"xfm_{ty}"][:])

            cw = {}
            for rname, kind, _, _ in RELS:
                cw[rname] = {}
                if kind == "gat":
                    cw[rname]["wl"] = ld3(cst, f"wl_{rname}_sb", dr[f"wl_{rname}"], H * D)
                    cw[rname]["wr"] = ld3(cst, f"wr_{rname}_sb", dr[f"wr_{rname}"], H * D)
                    cw[rname]["att"] = ld3(cst, f"att_{rname}_sb", dr[f"att_{rname}"], H * D)
                    cw[rname]["gb"] = ld3(cst, f"gb_{rname}_sb", dr[f"gb_{rname}"], D)
                else:
                    cw[rname]["wt"] = ld3(cst, f"wt_{rname}_sb", dr[f"wt_{rname}"], 2 * D)
                    cw[rname]["wb"] = ld3(cst, f"wb_{rname}_sb", dr[f"wb_{rname}"], 2 * D)
                    cbt = cst.tile([1, L * 2 * D], BF16, name=f"cb_{rname}_sb", tag=f"cb_{rname}_sb")
                    nc.sync.dma_start(
                        cbt[:].rearrange("p (l n) -> p l n", l=L),
                        dr[f"cb_{rname}"][:].rearrange("l p n -> p l n"),
                    )
                    cw[rname]["cb"] = cbt
            nw_w = ld3(cst, "nw_w_sb", dr["nw_w"], D)
            nw_b = ld3(cst, "nw_b_sb", dr["nw_b"], 1)
            ident_f = cst.tile([128, 128], F32, name="identf_sb", tag="identf_sb")
            nc.sync.dma_start(ident_f[:], dr["ident_f"][:])
            ident_b = cst.tile([128, 128], BF16, name="identb_sb", tag="identb_sb")
            nc.sync.dma_start(ident_b[:], dr["ident_b"][:])
            ones_b = cst.tile([1, 128], BF16, name="ones_sb", tag="ones_sb")
            nc.gpsimd.memset(ones_b[:], 1.0)

            cp_engines = [
                lambda o, i: nc.scalar.copy(o, i),
            ]

            # ---------------- layers
            for l in range(k_layers):
                acc_written = set()
                ACC = {}
                for ty in ("my", "opp"):
                    ACC[ty] = accp.tile([128, TILES * D], BF16, name=f"acc_{ty}_{l}", tag=f"acc_{ty}")

                for rname, kind, sty, dty in rels_active:
                    cwr = cw[rname]
                    bo = boff[rname]
                    W = H * D if kind == "gat" else 2 * D
                    cpi = 0
                    for g in range(NGR):
                        gt0 = g * TPG
                        gblk = bo[gt0 + TPG] - bo[gt0]
                        gepq = gblk * 128
                        geoff = bo[gt0] * 128
                        # -------- src gather for the 4-tile group
                        sit = sip.tile([128, NBG * 8], I16, name=f"si_{rname}_{l}_{g}", tag="sit")
                        nc.sync.dma_start(
                            sit[:, :gepq // 16],
                            dr[f"si_{rname}"][:, geoff // 16:(geoff + gepq) // 16])
                        xs = gth.tile([128, NBG * 128], BF16, name=f"xs_{rname}_{l}_{g}", tag="xs")
                        nc.gpsimd.dma_gather(
                            out_ap=xs[:, :gepq].rearrange("p (o n) -> p o n", o=1),
                            in_ap=xw[sty][:],
                            idxs_ap=sit[:, :gepq // 16],
                            num_idxs=gepq, num_idxs_reg=gepq,
                            elem_size=128, transpose=True,
                            single_packet=False,
                            sbuf_tokens_per_rank=128,
                            sbuf_free_dim_per_rank=256,
                            sbuf_free_dim_pad_per_rank=0,
                            sbuf_byte_offset=0,
                        )

                        for ci in range(TPG // TPC):
                            t0 = gt0 + ci * TPC
                            nblk = bo[t0 + TPC] - bo[t0]
                            epq = nblk * 128
                            eoff = bo[t0] * 128
                            xoff = eoff - geoff   # col offset into xs

                            ohA = ohp.tile([128, NBC * 128], BF16, name=f"ohA_{rname}_{l}_{t0}", tag="ohA")
                            nc.sync.dma_start(ohA[:, :epq], dr[f"ohA_{rname}"][:, eoff:eoff + epq])
                            ohS = ohp.tile([128, NBC * 128], BF16, name=f"ohS_{rname}_{l}_{t0}", tag="ohS")
                            nc.sync.dma_start(ohS[:, :epq], dr[f"ohS_{rname}"][:, eoff:eoff + epq])

                            # ---- per-tile dst transforms (xdW)
                            xdw = xdp.tile([128, TPC * H * D], BF16,
                                           name=f"xdw_{rname}_{l}_{t0}", tag="xdw")
                            for ti in range(TPC):
                                t = t0 + ti
                                pzx = pzp.tile([128, W], F32, name=f"pzx_{rname}_{l}_{t}", tag="pz")
                                if kind == "gat":
                                    nc.tensor.matmul(pzx[:], xfm[dty][:, t * D:(t + 1) * D],
                                                     cwr["wr"][:, l * W:(l + 1) * W],
                                                     start=True, stop=True)
                                else:
                                    nc.tensor.matmul(pzx[:], xfm[dty][:, t * D:(t + 1) * D],
                                                     cwr["wt"][:, l * W:(l + 1) * W],
                                                     start=True, stop=False)
                                    nc.tensor.matmul(pzx[:], ones_b[:],
                                                     cwr["cb"][:, l * W:(l + 1) * W],
                                                     start=False, stop=True)
                                cp_engines[0](xdw[:, ti * W:(ti + 1) * W], pzx[:])
                                cpi += 1

                            # ---- per-block edge transforms -> praw / m_in
                            praw = prp.tile([128, NBC * H * D], BF16,
                                            name=f"praw_{rname}_{l}_{t0}", tag="praw")
                            for ti in range(TPC):
                                t = t0 + ti
                                nb = BPT[rname][t]
                                bof = bo[t] - bo[t0]
                                for b in range(nb):
                                    co = (bof + b) * 128
                                    psz = pzp.tile([128, W], F32, name=f"psz_{rname}_{l}_{t}_{b}", tag="pz")
                                    nc.tensor.matmul(psz[:], xs[:, xoff + co:xoff + co + 128],
                                                     cwr["wl" if kind == "gat" else "wb"][:, l * W:(l + 1) * W],
                                                     start=True, stop=False)
                                    nc.tensor.matmul(psz[:], ohS[:, co:co + 128],
                                                     xdw[:, ti * W:(ti + 1) * W],
                                                     start=False, stop=True)
                                    cp_engines[0](praw[:, (bof + b) * W:(bof + b + 1) * W], psz[:])
                                    cpi += 1

                            if kind == "gat":
                                # ---- scores: z = prelu(praw); sc = reduce(z*att)
                                sc = wrk.tile([128, NBC * H], F32, name=f"sc_{rname}_{l}_{t0}", tag="sc")
                                for ti in range(TPC):
                                    t = t0 + ti
                                    nb = BPT[rname][t]
                                    bof = bo[t] - bo[t0]
                                    zt = zp.tile([128, 5 * W], BF16, name=f"z_{rname}_{l}_{t}", tag="z")
                                    zv = zt[:, :nb * W]
                                    pv = praw[:, bof * W:(bof + nb) * W]
                                    nc.scalar.activation(zv, pv, AF.Prelu, alpha=0.2)
                                    attv = cwr["att"][:, l * W:(l + 1) * W]
                                    zv3 = zv.rearrange("p (b f) -> p b f", f=W)
                                    nc.vector.tensor_tensor(
                                        zv3, zv3, attv.unsqueeze(1).broadcast_to([128, nb, W]),
                                        op=OP.mult)
                                    nc.vector.tensor_reduce(
                                        sc[:, bof * H:(bof + nb) * H],
                                        zv.rearrange("p (bh f) -> p bh f", f=D),
                                        axis=mybir.AxisListType.X, op=OP.add)
                                es = wrk.tile([128, NBC * H], BF16, name=f"es_{rname}_{l}_{t0}", tag="es")
                                nc.scalar.activation(es[:, :nblk * H], sc[:, :nblk * H], AF.Exp)
                                # ---- alpha-weighted psz + aggregation
                                for ti in range(TPC):
                                    t = t0 + ti
                                    nb = BPT[rname][t]
                                    bof = bo[t] - bo[t0]
                                    xv = praw[:, bof * W:(bof + nb) * W].rearrange(
                                        "p (bh f) -> p bh f", f=D)
                                    nc.vector.tensor_tensor(
                                        xv, xv,
                                        es[:, bof * H:(bof + nb) * H].unsqueeze(2)
                                        .broadcast_to([128, nb * H, D]),
                                        op=OP.mult)
                                    pagg = paggp.tile([128, W], F32, name=f"pagg_{rname}_{l}_{t}", tag="pagg")
                                    psum_s = pssp.tile([128, H], F32, name=f"psums_{rname}_{l}_{t}", tag="pss")
                                    for b in range(nb):
                                        co = (bof + b) * 128
                                        first, last = (b == 0), (b == nb - 1)
                                        nc.tensor.matmul(pagg[:], ohA[:, co:co + 128],
                                                         praw[:, (bof + b) * W:(bof + b + 1) * W],
                                                         start=first, stop=last)
                                        nc.tensor.matmul(psum_s[:], ohA[:, co:co + 128],
                                                         es[:, (bof + b) * H:(bof + b + 1) * H],
                                                         start=first, stop=last)
                                    # ---- tile epilogue:
                                    # out_h = (pagg_h - xrW_h*s_h)/(4(s_h+eps)); sum_h; +bias
                                    asl = ACC[dty][:, t * D:(t + 1) * D]
                                    inv4 = wrk.tile([128, H], F32, name=f"inv4_{rname}_{l}_{t}", tag="inv4")
                                    nc.vector.tensor_scalar(inv4[:], psum_s[:], 1e-16, 4.0,
                                                            op0=OP.add, op1=OP.mult)
                                    nc.vector.reciprocal(inv4[:], inv4[:])
                                    nsw = wrk.tile([128, H], F32, name=f"nsw_{rname}_{l}_{t}", tag="nsw")
                                    nc.vector.tensor_tensor(nsw[:], psum_s[:], inv4[:], op=OP.mult)
                                    nc.vector.tensor_scalar(nsw[:], nsw[:], -1.0, None, op0=OP.mult)
                                    # xdw_tile *= -s*inv4 (per head), gw = pagg*inv4;
                                    # head-sum via 2-step tree; + bias
                                    xv = xdw[:, ti * W:(ti + 1) * W].rearrange("p (h f) -> p h f", f=D)
                                    nc.vector.tensor_tensor(
                                        xv, xv, nsw[:].unsqueeze(2).broadcast_to([128, H, D]),
                                        op=OP.mult)
                                    gw = wrk.tile([128, W], BF16, name=f"gw_{rname}_{l}_{t}", tag="gw")
                                    nc.vector.tensor_tensor(
                                        gw[:].rearrange("p (h f) -> p h f", f=D),
                                        pagg[:].rearrange("p (h f) -> p h f", f=D),
                                        inv4[:].unsqueeze(2).broadcast_to([128, H, D]),
                                        op=OP.mult)
                                    nc.vector.tensor_tensor(gw[:], gw[:], xdw[:, ti * W:(ti + 1) * W], op=OP.add)
                                    nc.vector.tensor_tensor(gw[:, 0:2 * D], gw[:, 0:2 * D], gw[:, 2 * D:4 * D], op=OP.add)
                                    nc.vector.tensor_tensor(gw[:, 0:D], gw[:, 0:D], gw[:, D:2 * D], op=OP.add)
                                    nc.vector.tensor_tensor(gw[:, 0:D], gw[:, 0:D],
                                                            cwr["gb"][:, l * D:(l + 1) * D], op=OP.add)
                                    if (dty, t) in acc_written:
                                        nc.vector.tensor_tensor(asl, asl, gw[:, 0:D], op=OP.add)
                                    else:
                                        nc.vector.tensor_copy(asl, gw[:, 0:D])
                                    acc_written.add((dty, t))
                            else:
                                # ---- batched CG gate/softplus chain
                                mi = praw[:, :nblk * W].rearrange("p (n c) -> p n c", c=W)
                                gatev = mi[:, :, 0:D]
                                softv = mi[:, :, D:2 * D]
                                sgA = cgs.tile([128, NBC * D], F32, name=f"sgA_{rname}_{l}_{t0}", tag="sgA")
                                spB = cgs.tile([128, NBC * D], BF16, name=f"spB_{rname}_{l}_{t0}", tag="spB")
                                m_out = cgs.tile([128, NBC * D], BF16, name=f"mout_{rname}_{l}_{t0}", tag="mout")
                                Av = sgA[:, :nblk * D].rearrange("p (n c) -> p n c", c=D)
                                nc.scalar.activation(Av, gatev, AF.Exp, scale=-1.0)
                                nc.scalar.activation(
                                    spB[:, :nblk * D].rearrange("p (n c) -> p n c", c=D),
                                    softv, AF.Exp)
                                nc.scalar.activation(m_out[:, :nblk * D], spB[:, :nblk * D],
                                                     AF.Ln, bias=1.0)
                                nc.gpsimd.tensor_scalar(sgA[:, :nblk * D], sgA[:, :nblk * D],
                                                        1.0, None, op0=OP.add)
                                nc.vector.reciprocal_approx_fast(sgA[:, :nblk * D], sgA[:, :nblk * D])
                                nc.gpsimd.tensor_tensor(m_out[:, :nblk * D], m_out[:, :nblk * D],
                                                        sgA[:, :nblk * D], op=OP.mult)
                                # ---- aggregation + residual
                                for ti in range(TPC):
                                    t = t0 + ti
                                    nb = BPT[rname][t]
                                    bof = bo[t] - bo[t0]
                                    pagg = paggp.tile([128, H * D], F32, name=f"paggc_{rname}_{l}_{t}", tag="pagg")
                                    for b in range(nb):
                                        co = (bof + b) * 128
                                        nc.tensor.matmul(pagg[:, 0:D], ohA[:, co:co + 128],
                                                         m_out[:, (bof + b) * D:(bof + b + 1) * D],
                                                         start=(b == 0), stop=(b == nb - 1))
                                    asl = ACC[dty][:, t * D:(t + 1) * D]
                                    if (dty, t) in acc_written:
                                        nc.vector.tensor_tensor(asl, asl, pagg[:, 0:D], op=OP.add)
                                        nc.vector.tensor_tensor(
                                            asl, asl, xres[dty][:, t * D:(t + 1) * D], op=OP.add)
                                    else:
                                        nc.vector.scalar_tensor_tensor(
                                            asl, pagg[:, 0:D], 1.0, xres[dty][:, t * D:(t + 1) * D],
                                            op0=OP.mult, op1=OP.add)
                                    acc_written.add((dty, t))

                # ---------------- layer epilogue: nodewise linear + layout
                last_layer = (l == k_layers - 1)
                ag_in, ag_out = {}, {}
                if not last_layer:
                    for ty in ("my", "opp"):
                        ag_in[ty] = drm.tile([128, TILES * D], BF16,
                                             name=f"agin_{ty}_{l}", tag=f"agin_{ty}")
                        ag_out[ty] = drm.tile([CORES * 128, TILES * D], BF16,
                                              name=f"agout_{ty}_{l}", tag=f"agout_{ty}",
                                              addr_space="Shared")
                for tyi, ty in enumerate(("my", "opp")):
                    if ty not in {r[3] for r in rels_active}:
                        continue
                    for k in range(TILES // TPG):   # 512-col groups
                        accT = epi.tile([128, TPG * D], BF16, name=f"accT_{ty}_{l}_{k}", tag="accT")
                        for j in range(TPG):
                            t = k * TPG + j
                            ptr = psp.tile([128, 128], BF16, name=f"ptr_{ty}_{l}_{t}", tag="ps")
                            nc.tensor.transpose(ptr[:], ACC[ty][:, t * D:(t + 1) * D], ident_b[:])
                            if j % 2 == 0:
                                nc.scalar.copy(accT[:, j * D:(j + 1) * D], ptr[:])
                            else:
                                nc.vector.tensor_copy(accT[:, j * D:(j + 1) * D], ptr[:])
                        pnw = paggp.tile([128, TPG * D], F32, name=f"pnw_{ty}_{l}_{k}", tag="pagg")
                        nc.tensor.matmul(pnw[:], nw_w[:, l * D:(l + 1) * D], accT[:],
                                         start=True, stop=True)
                        if not last_layer:
                            xnk = xfm[ty][:, k * TPG * D:(k + 1) * TPG * D]
                            nc.scalar.activation(xnk, pnw[:], AF.Identity, bias=nw_b[:, l:l + 1])
                            for j in range(TPG):
                                t = k * TPG + j
                                ptr2 = psp.tile([128, 128], BF16, name=f"ptr2_{ty}_{l}_{t}", tag="ps")
                                nc.tensor.transpose(ptr2[:], xfm[ty][:, t * D:(t + 1) * D], ident_b[:])
                                nc.vector.tensor_copy(xres[ty][:, t * D:(t + 1) * D], ptr2[:])
                        else:
                            xnk = epi.tile([128, TPG * D], F32, name=f"xnT_{ty}_{l}_{k}", tag="xnT")
                            nc.scalar.activation(xnk[:], pnw[:], AF.Identity, bias=nw_b[:, l:l + 1])
                            for j in range(TPG):
                                t = k * TPG + j
                                ptr2 = psp.tile([128, 128], F32, name=f"ptr2_{ty}_{l}_{t}", tag="ps")
                                nc.tensor.transpose(ptr2[:], xnk[:, j * D:(j + 1) * D], ident_f[:])
                                osb = epi.tile([128, 128], F32, name=f"osb_{ty}_{l}_{t}", tag="osb")
                                nc.vector.tensor_copy(osb[:], ptr2[:])
                                nc.sync.dma_start(dr[f"out_{ty}"][t * 128:(t + 1) * 128, :], osb[:])
                    if not last_layer:
                        nc.sync.dma_start(ag_in[ty][:], xres[ty][:])
                        nc.gpsimd.collective_compute(
                            "AllGather", mybir.AluOpType.bypass,
                            replica_groups=[list(range(CORES))],
                            ins=[ag_in[ty].opt()], outs=[ag_out[ty].opt()],
                        )
                        nc.sync.dma_start(
                            xw[ty][:].rearrange("p (c j) -> p c j", c=CORES),
                            ag_out[ty][:].rearrange("(c p) j -> p c j", p=128),
                        )

    nc.compile()
    return nc


_prog_cache = {}


def _get_program(cfg):
    if cfg not in _prog_cache:
        _prog_cache[cfg] = _build_program(cfg)
    return _prog_cache[cfg]


# ------------------------------------------------------------------- kernel

def kernel(**inputs):
    global LAST_EXEC_NS
    from concourse.bass_utils import run_bass_kernel_spmd

    f32 = lambda k: np.asarray(inputs[k], np.float32)
    xpad = {}
    for ty, key in (("my", "x_my"), ("opp", "x_opp")):
        xp = np.zeros((NPAD, D), np.float32)
        xp[:N] = f32(key)
        xpad[ty] = xp

    BPT, packed, pos_g, order_ids = _prep_graph(inputs)
    cfg = tuple(tuple(int(v) for v in BPT[r[0]]) for r in RELS)
    nc = _get_program(cfg)

    # shared (per-core identical) tensors
    shared = {}
    for rname, kind, _, _ in RELS:
        tag = {"loses": "cg_lose", "beats": "gat_beats",
               "rev_beats": "cg_rev", "rev_loses": "gat_rev"}[rname]
        if kind == "gat":
            shared[f"wl_{rname}"] = np.ascontiguousarray(f32(f"{tag}_Wl")).astype(BF)
            shared[f"wr_{rname}"] = np.ascontiguousarray(f32(f"{tag}_Wr")).astype(BF)
            att = f32(f"{tag}_att")
            shared[f"att_{rname}"] = np.stack(
                [_rep(att[l].reshape(-1)) for l in range(L)]).astype(BF)
            b = f32(f"{tag}_b")
            shared[f"gb_{rname}"] = np.stack([_rep(b[l]) for l in range(L)])
        else:
            wf, ws = f32(f"{tag}_Wf"), f32(f"{tag}_Ws")
            shared[f"wt_{rname}"] = np.ascontiguousarray(
                np.concatenate([wf[:, :D, :], ws[:, :D, :]], axis=2)).astype(BF)
            shared[f"wb_{rname}"] = np.ascontiguousarray(
                np.concatenate([wf[:, D:, :], ws[:, D:, :]], axis=2)).astype(BF)
            bfv, bsv = f32(f"{tag}_bf"), f32(f"{tag}_bs")
            shared[f"cb_{rname}"] = np.ascontiguousarray(
                np.concatenate([bfv, bsv], axis=1).reshape(L, 1, 2 * D)).astype(BF)
    shared["nw_w"] = np.ascontiguousarray(f32("nw_W")).astype(BF)
    shared["nw_b"] = np.ascontiguousarray(f32("nw_b").reshape(L, 128, 1))
    shared["ident_f"] = np.eye(128, dtype=np.float32)
    shared["ident_b"] = np.eye(128).astype(BF)
    for ty in ("my", "opp"):
        inv = np.empty(NPAD, np.int64)
        inv[pos_g[ty]] = np.arange(NPAD)
        shared[f"xw_{ty}"] = _wrap_nodes(xpad[ty][inv])

    in_maps = []
    for c in range(CORES):
        m = dict(shared)
        for ty in ("my", "opp"):
            loc = xpad[ty][c * SHARD:(c + 1) * SHARD][order_ids[(ty, c)]]
            m[f"xres_{ty}"] = np.ascontiguousarray(
                loc.reshape(TILES, 128, D).transpose(1, 0, 2).reshape(128, TILES * D)
            ).astype(BF)
            m[f"xfm_{ty}"] = np.ascontiguousarray(loc.T).astype(BF)
        for rname, kind, _, _ in RELS:
            si, ohA, ohS = packed[(rname, c)]
            m[f"si_{rname}"] = si
            m[f"ohA_{rname}"] = ohA
            m[f"ohS_{rname}"] = ohS
        in_maps.append(m)

    trace = os.environ.get("KERNEL_PROFILE", "0") == "1"
    res = run_bass_kernel_spmd(nc, in_maps, core_ids=list(range(CORES)),
                               trace=trace, trace_cores=[0] if trace else None)
    LAST_EXEC_NS = res.exec_time_ns

    out = {}
    for ty in ("my", "opp"):
        full = np.concatenate([res.results[c][f"out_{ty}"] for c in range(CORES)])
        out[ty] = full[pos_g[ty][:N]]
    return out["my"], out["opp"]


# revision 17
# speedup vs baseline: 1.4083x; 1.4083x over previous
"""Trainium2 Bass kernel for the 2-layer heterogeneous GNN (GATv2 + CGConv).

Sharding: destination nodes (both node types) are split into 8 contiguous
ranges of 2560 (N padded 20000 -> 20480); each core owns the edges that
target its range, for all 4 relations.  Node features are replicated
(SBUF-resident, bf16, node-wrapped layout); the one inter-layer halo
exchange is a single AllGather of the updated 2560-row slices.

Within a core, dst nodes are PERMUTED into 20 tiles of 128 so that each
tile's incoming edge count (per relation) is balanced -- most tiles then
need only 4 edge-blocks of 128 (vs 5 unbalanced).  One-hot edge->dst
selectors are built on the host and streamed from DRAM in two orientations:
  oh_agg [e_p, d_f]  - lhsT of the aggregation matmul (segment sum)
  oh_sel [d_p, e_f]  - lhsT of the dst-feature-select matmul, which
                       replaces the dst-side per-edge gather entirely:
                       psz[e,:] += oh_sel^T @ (x_dst_tile @ W_dst).
Only the src side is gathered (gpsimd dma_gather, feature-major).

GATv2 aggregates the alpha-weighted SUM tensor psz = xl + xr directly and
recovers sum(alpha*xl) as (sum(es*psz) - xrW*s) / s per dst (softmax
weights sum to s per dst), which removes the per-block mid-PSUM copy of
xl.  Per-edge elementwise work is batched per 2-tile chunk (broadcast APs
for the alpha scaling, strided views for the CGConv gate/softplus halves),
with activation functions grouped to avoid ACT table reloads.
"""

import os
import numpy as np
import ml_dtypes

BF = ml_dtypes.bfloat16

N = 20000
D = 128
H = 4
L = 2
E = 80000
CORES = 8
NPAD = 20480
SHARD = 2560
TILES = 20           # dst tiles of 128 per core
TPG = 4              # tiles per gather group
TPC = 2              # tiles per compute chunk
NGR = TILES // TPG   # gather groups per relation (5)
RANKS = NPAD // 128  # 160
PAD_NODE = 20000     # zero-feature padding node (valid gather target)

LAST_EXEC_NS = None

# relation table: (name, kind, src_type, dst_type); CG first per dst type
RELS = [
    ("loses", "cg", "my", "opp"),
    ("beats", "gat", "my", "opp"),
    ("rev_beats", "cg", "opp", "my"),
    ("rev_loses", "gat", "opp", "my"),
]


# ----------------------------------------------------------------- host prep

def _balance(c1, c2):
    """Assign 2560 local nodes to 20 tiles of 128, balancing both relations'
    per-tile edge counts.  Tiles are then sorted heaviest-first so that the
    per-tile-index block counts align across cores (BPT is maxed over
    cores).  Returns assign[2560] -> tile id."""
    order = np.argsort(-np.maximum(c1, c2), kind="stable")
    l1 = np.zeros(TILES, np.int64)
    l2 = np.zeros(TILES, np.int64)
    cnt = np.zeros(TILES, np.int64)
    assign = np.empty(SHARD, np.int64)
    # skewed capacity profile: surplus beyond 20*512 concentrates in the
    # leading tiles, which later align across cores (heaviest-first order)
    capv = np.full(TILES, 4 * 128, np.float64)
    capv[0], capv[1], capv[2] = 6 * 128, 5 * 128, 5 * 128
    for n in order:
        m1, m2 = l1 + c1[n], l2 + c2[n]
        f = np.maximum(np.maximum(m1 / capv, m2 / capv), (cnt + 1) / 128.0)
        ok = (m1 <= capv) & (m2 <= capv) & (cnt < 128)
        if ok.any():
            f = np.where(ok, f, np.inf)
        else:
            f = np.maximum(m1, m2).astype(np.float64)
            f[cnt >= 128] = np.inf
        t = int(np.argmin(f))
        assign[n] = t
        l1[t] += c1[n]
        l2[t] += c2[n]
        cnt[t] += 1
    # heaviest-first tile ordering (aligns block counts across cores)
    n1 = -(-l1 // 128)
    n2 = -(-l2 // 128)
    key = (n1 + n2) * 10000 + np.maximum(n1, n2) * 100
    rank = np.empty(TILES, np.int64)
    rank[np.argsort(-key, kind="stable")] = np.arange(TILES)
    return rank[assign]


def _wrap_nodes(xg):
    """position-ordered features [NPAD, D] f32 -> node-wrapped
    [128, RANKS*D] bf16 (position p at partition p%128, cols (p//128)*D)."""
    return np.ascontiguousarray(
        xg.reshape(RANKS, 128, D).transpose(1, 0, 2).reshape(128, RANKS * D)
    ).astype(BF)


def _idx_dev(a):
    """[EP] int -> [128, EP//16] int16 (16-partition wrap, replicated 8x)."""
    x = a.astype(np.int16).reshape(-1, 16).T
    return np.ascontiguousarray(np.tile(x, (8, 1)))


def _rep(v, rows=128):
    return np.ascontiguousarray(
        np.tile(np.asarray(v, np.float32).reshape(1, -1), (rows, 1)))


def _prep_graph(inputs):
    """Permutations, per-core packed edges, one-hots, BPT config."""
    ei = {r: np.asarray(inputs[k]).astype(np.int64)
          for r, k in (("loses", "ei_loses"), ("beats", "ei_beats"),
                       ("rev_beats", "ei_rev_beats"),
                       ("rev_loses", "ei_rev_loses"))}
    rels_of_type = {"opp": ("beats", "loses"), "my": ("rev_beats", "rev_loses")}

    assign = {}      # (ty, c) -> [SHARD] tile id
    pos_local = {}   # (ty, c) -> [SHARD] permuted position within shard
    order_ids = {}   # (ty, c) -> [SHARD] local node id at each position
    for ty, (r1, r2) in rels_of_type.items():
        d1 = np.bincount(ei[r1][1], minlength=NPAD)
        d2 = np.bincount(ei[r2][1], minlength=NPAD)
        for c in range(CORES):
            sl = slice(c * SHARD, (c + 1) * SHARD)
            a = _balance(d1[sl], d2[sl])
            assign[(ty, c)] = a
            order = np.argsort(a * SHARD + np.arange(SHARD), kind="stable")
            order_ids[(ty, c)] = order
            p = np.empty(SHARD, np.int64)
            p[order] = np.arange(SHARD)
            pos_local[(ty, c)] = p

    pos_g = {}
    for ty in ("my", "opp"):
        pg = np.empty(NPAD, np.int64)
        for c in range(CORES):
            pg[c * SHARD:(c + 1) * SHARD] = c * SHARD + pos_local[(ty, c)]
        pos_g[ty] = pg

    # per-tile block counts, maxed over cores (program must be SPMD-uniform)
    BPT = {r: np.zeros(TILES, np.int64) for r in ei}
    edges = {}
    for rname, kind, sty, dty in RELS:
        src, dst = ei[rname]
        for c in range(CORES):
            m = (dst >= c * SHARD) & (dst < (c + 1) * SHARD)
            s, d = src[m], dst[m] - c * SHARD
            t = assign[(dty, c)][d]
            r = pos_local[(dty, c)][d] % 128
            edges[(rname, c)] = (s, t, r)
            cnt = np.bincount(t, minlength=TILES)
            BPT[rname] = np.maximum(BPT[rname], -(-cnt // 128))
        BPT[rname] = np.maximum(BPT[rname], 1)

    packed = {}
    for rname, kind, sty, dty in RELS:
        bpt = BPT[rname]
        boff = np.concatenate([[0], np.cumsum(bpt)])
        EP = int(boff[-1]) * 128
        for c in range(CORES):
            s, t, r = edges[(rname, c)]
            o = np.argsort(t, kind="stable")
            s, t, r = s[o], t[o], r[o]
            cnts = np.bincount(t, minlength=TILES)
            idx_in_tile = np.concatenate(
                [np.arange(cnts[tt]) for tt in range(TILES)])
            slot = boff[t] * 128 + idx_in_tile
            si_pos = np.full(EP, pos_g[sty][PAD_NODE], np.int64)
            si_pos[slot] = pos_g[sty][s]
            ohf = np.zeros((EP, 128), np.float32)
            ohf[slot, r] = 1.0
            ohb = ohf.reshape(-1, 128, 128)
            oh_agg = np.ascontiguousarray(
                ohb.transpose(1, 0, 2).reshape(128, EP)).astype(BF)
            oh_sel = np.ascontiguousarray(
                ohb.transpose(2, 0, 1).reshape(128, EP)).astype(BF)
            packed[(rname, c)] = (_idx_dev(si_pos), oh_agg, oh_sel)

    return BPT, packed, pos_g, order_ids


# ------------------------------------------------------------- program build

def _build_program(cfg):
    import concourse.bass as bass
    import concourse.bacc as bacc
    import concourse.mybir as mybir
    import concourse.tile as tile

    F32, BF16, I16 = mybir.dt.float32, mybir.dt.bfloat16, mybir.dt.int16
    AF = mybir.ActivationFunctionType
    OP = mybir.AluOpType

    BPT = {r[0]: list(v) for r, v in zip(RELS, cfg)}
    boff = {}
    for r in BPT:
        boff[r] = [0]
        for t in range(TILES):
            boff[r].append(boff[r][t] + BPT[r][t])
    EP = {r: boff[r][-1] * 128 for r in BPT}
    # max blocks per compute chunk / gather group
    NBC = max(boff[r][t + TPC] - boff[r][t]
              for r in BPT for t in range(0, TILES, TPC))
    NBG = max(boff[r][t + TPG] - boff[r][t]
              for r in BPT for t in range(0, TILES, TPG))

    k_layers = int(os.environ.get("K_LAYERS", str(L)))
    k_rels = os.environ.get("K_RELS", "")
    rels_active = [r for r in RELS if (not k_rels or r[0] in k_rels.split(","))]

    nc = bacc.Bacc("TRN2", target_bir_lowering=False, debug=False,
                   num_devices=CORES)

    dr = {}
    for ty in ("my", "opp"):
        dr[f"xw_{ty}"] = nc.dram_tensor(f"xw_{ty}", [128, RANKS * D], BF16,
                                        kind="ExternalInput")
        dr[f"xres_{ty}"] = nc.dram_tensor(f"xres_{ty}", [128, TILES * D], BF16,
                                          kind="ExternalInput")
        dr[f"xfm_{ty}"] = nc.dram_tensor(f"xfm_{ty}", [128, TILES * D], BF16,
                                         kind="ExternalInput")
    for rname, kind, _, _ in RELS:
        dr[f"si_{rname}"] = nc.dram_tensor(
            f"si_{rname}", [128, EP[rname] // 16], I16, kind="ExternalInput")
        dr[f"ohA_{rname}"] = nc.dram_tensor(
            f"ohA_{rname}", [128, EP[rname]], BF16, kind="ExternalInput")
        dr[f"ohS_{rname}"] = nc.dram_tensor(
            f"ohS_{rname}", [128, EP[rname]], BF16, kind="ExternalInput")
        if kind == "gat":
            dr[f"wl_{rname}"] = nc.dram_tensor(f"wl_{rname}", [L, 128, H * D], BF16, kind="ExternalInput")
            dr[f"wr_{rname}"] = nc.dram_tensor(f"wr_{rname}", [L, 128, H * D], BF16, kind="ExternalInput")
            dr[f"att_{rname}"] = nc.dram_tensor(f"att_{rname}", [L, 128, H * D], BF16, kind="ExternalInput")
            dr[f"gb_{rname}"] = nc.dram_tensor(f"gb_{rname}", [L, 128, D], F32, kind="ExternalInput")
        else:
            dr[f"wt_{rname}"] = nc.dram_tensor(f"wt_{rname}", [L, 128, 2 * D], BF16, kind="ExternalInput")
            dr[f"wb_{rname}"] = nc.dram_tensor(f"wb_{rname}", [L, 128, 2 * D], BF16, kind="ExternalInput")
            dr[f"cb_{rname}"] = nc.dram_tensor(f"cb_{rname}", [L, 1, 2 * D], BF16, kind="ExternalInput")
    dr["nw_w"] = nc.dram_tensor("nw_w", [L, 128, D], BF16, kind="ExternalInput")
    dr["nw_b"] = nc.dram_tensor("nw_b", [L, 128, 1], F32, kind="ExternalInput")
    dr["ident_f"] = nc.dram_tensor("ident_f", [128, 128], F32, kind="ExternalInput")
    dr["ident_b"] = nc.dram_tensor("ident_b", [128, 128], BF16, kind="ExternalInput")
    dr["out_my"] = nc.dram_tensor("out_my", [SHARD, D], F32, kind="ExternalOutput")
    dr["out_opp"] = nc.dram_tensor("out_opp", [SHARD, D], F32, kind="ExternalOutput")

    def ld3(pool, name, src, cols):
        t = pool.tile([128, L * cols], src.dtype, name=name, tag=name)
        nc.sync.dma_start(
            t[:].rearrange("p (l n) -> p l n", l=L),
            src[:].rearrange("l p n -> p l n"),
        )
        return t

    from contextlib import ExitStack

    with tile.TileContext(nc) as tc:
        with ExitStack() as _st:
            cst = _st.enter_context(tc.tile_pool(name="const", bufs=1))
            xwp = _st.enter_context(tc.tile_pool(name="xwp", bufs=1))
            accp = _st.enter_context(tc.tile_pool(name="accp", bufs=1))
            gth = _st.enter_context(tc.tile_pool(name="gth", bufs=2))
            sip = _st.enter_context(tc.tile_pool(name="sip", bufs=2))
            ohp = _st.enter_context(tc.tile_pool(name="ohp", bufs=2))
            xdp = _st.enter_context(tc.tile_pool(name="xdp", bufs=2))
            prp = _st.enter_context(tc.tile_pool(name="prp", bufs=2))
            zp = _st.enter_context(tc.tile_pool(name="zp", bufs=2))
            wrk = _st.enter_context(tc.tile_pool(name="wrk", bufs=2))
            cgs = _st.enter_context(tc.tile_pool(name="cgs", bufs=2))
            epi = _st.enter_context(tc.tile_pool(name="epi", bufs=2))
            drm = _st.enter_context(tc.tile_pool(name="dram", bufs=1, space="DRAM"))
            pzp = _st.enter_context(tc.tile_pool(name="pz", bufs=2, space=bass.MemorySpace.PSUM))
            paggp = _st.enter_context(tc.tile_pool(name="pagg", bufs=2, space=bass.MemorySpace.PSUM))
            pssp = _st.enter_context(tc.tile_pool(name="pss", bufs=2, space=bass.MemorySpace.PSUM))
            psp = _st.enter_context(tc.tile_pool(name="ps", bufs=2, space=bass.MemorySpace.PSUM))

            # ---------------- constants / inputs resident in SBUF
            xw, xres, xfm = {}, {}, {}
            for ty in ("my", "opp"):
                xw[ty] = xwp.tile([128, RANKS * D], BF16, name=f"xw_{ty}_sb", tag=f"xw_{ty}_sb")
                nc.sync.dma_start(xw[ty][:], dr[f"xw_{ty}"][:])
                xres[ty] = xwp.tile([128, TILES * D], BF16, name=f"xres_{ty}_sb", tag=f"xres_{ty}_sb")
                nc.sync.dma_start(xres[ty][:], dr[f"xres_{ty}"][:])
                xfm[ty] = xwp.tile([128, TILES * D], BF16, name=f"xfm_{ty}_sb", tag=f"xfm_{ty}_sb")
                nc.sync.dma_start(xfm[ty][:], dr[f"xfm_{ty}"][:])

            cw = {}
            for rname, kind, _, _ in RELS:
                cw[rname] = {}
                if kind == "gat":
                    cw[rname]["wl"] = ld3(cst, f"wl_{rname}_sb", dr[f"wl_{rname}"], H * D)
                    cw[rname]["wr"] = ld3(cst, f"wr_{rname}_sb", dr[f"wr_{rname}"], H * D)
                    cw[rname]["att"] = ld3(cst, f"att_{rname}_sb", dr[f"att_{rname}"], H * D)
                    cw[rname]["gb"] = ld3(cst, f"gb_{rname}_sb", dr[f"gb_{rname}"], D)
                else:
                    cw[rname]["wt"] = ld3(cst, f"wt_{rname}_sb", dr[f"wt_{rname}"], 2 * D)
                    cw[rname]["wb"] = ld3(cst, f"wb_{rname}_sb", dr[f"wb_{rname}"], 2 * D)
                    cbt = cst.tile([1, L * 2 * D], BF16, name=f"cb_{rname}_sb", tag=f"cb_{rname}_sb")
                    nc.sync.dma_start(
                        cbt[:].rearrange("p (l n) -> p l n", l=L),
                        dr[f"cb_{rname}"][:].rearrange("l p n -> p l n"),
                    )
                    cw[rname]["cb"] = cbt
            nw_w = ld3(cst, "nw_w_sb", dr["nw_w"], D)
            nw_b = ld3(cst, "nw_b_sb", dr["nw_b"], 1)
            ident_f = cst.tile([128, 128], F32, name="identf_sb", tag="identf_sb")
            nc.sync.dma_start(ident_f[:], dr["ident_f"][:])
            ident_b = cst.tile([128, 128], BF16, name="identb_sb", tag="identb_sb")
            nc.sync.dma_start(ident_b[:], dr["ident_b"][:])
            ones_b = cst.tile([1, 128], BF16, name="ones_sb", tag="ones_sb")
            nc.gpsimd.memset(ones_b[:], 1.0)

            cp_engines = [
                lambda o, i: nc.scalar.copy(o, i),
            ]

            # ---------------- layers
            for l in range(k_layers):
                acc_written = set()
                ACC = {}
                for ty in ("my", "opp"):
                    ACC[ty] = accp.tile([128, TILES * D], BF16, name=f"acc_{ty}_{l}", tag=f"acc_{ty}")

                for rname, kind, sty, dty in rels_active:
                    cwr = cw[rname]
                    bo = boff[rname]
                    W = H * D if kind == "gat" else 2 * D
                    cpi = 0
                    for g in range(NGR):
                        gt0 = g * TPG
                        gblk = bo[gt0 + TPG] - bo[gt0]
                        gepq = gblk * 128
                        geoff = bo[gt0] * 128
                        # -------- src gather for the 4-tile group
                        sit = sip.tile([128, NBG * 8], I16, name=f"si_{rname}_{l}_{g}", tag="sit")
                        nc.sync.dma_start(
                            sit[:, :gepq // 16],
                            dr[f"si_{rname}"][:, geoff // 16:(geoff + gepq) // 16])
                        xs = gth.tile([128, NBG * 128], BF16, name=f"xs_{rname}_{l}_{g}", tag="xs")
                        nc.gpsimd.dma_gather(
                            out_ap=xs[:, :gepq].rearrange("p (o n) -> p o n", o=1),
                            in_ap=xw[sty][:],
                            idxs_ap=sit[:, :gepq // 16],
                            num_idxs=gepq, num_idxs_reg=gepq,
                            elem_size=128, transpose=True,
                            single_packet=False,
                            sbuf_tokens_per_rank=128,
                            sbuf_free_dim_per_rank=256,
                            sbuf_free_dim_pad_per_rank=0,
                            sbuf_byte_offset=0,
                        )

                        for ci in range(TPG // TPC):
                            t0 = gt0 + ci * TPC
                            nblk = bo[t0 + TPC] - bo[t0]
                            epq = nblk * 128
                            eoff = bo[t0] * 128
                            xoff = eoff - geoff   # col offset into xs

                            ohA = ohp.tile([128, NBC * 128], BF16, name=f"ohA_{rname}_{l}_{t0}", tag="ohA")
                            nc.sync.dma_start(ohA[:, :epq], dr[f"ohA_{rname}"][:, eoff:eoff + epq])
                            ohS = ohp.tile([128, NBC * 128], BF16, name=f"ohS_{rname}_{l}_{t0}", tag="ohS")
                            nc.sync.dma_start(ohS[:, :epq], dr[f"ohS_{rname}"][:, eoff:eoff + epq])

                            # ---- per-tile dst transforms (xdW)
                            xdw = xdp.tile([128, TPC * H * D], BF16,
                                           name=f"xdw_{rname}_{l}_{t0}", tag="xdw")
                            for ti in range(TPC):
                                t = t0 + ti
                                pzx = pzp.tile([128, W], F32, name=f"pzx_{rname}_{l}_{t}", tag="pz")
                                if kind == "gat":
                                    nc.tensor.matmul(pzx[:], xfm[dty][:, t * D:(t + 1) * D],
                                                     cwr["wr"][:, l * W:(l + 1) * W],
                                                     start=True, stop=True)
                                else:
                                    nc.tensor.matmul(pzx[:], xfm[dty][:, t * D:(t + 1) * D],
                                                     cwr["wt"][:, l * W:(l + 1) * W],
                                                     start=True, stop=False)
                                    nc.tensor.matmul(pzx[:], ones_b[:],
                                                     cwr["cb"][:, l * W:(l + 1) * W],
                                                     start=False, stop=True)
                                cp_engines[0](xdw[:, ti * W:(ti + 1) * W], pzx[:])
                                cpi += 1

                            # ---- per-block edge transforms -> praw / m_in
                            praw = prp.tile([128, NBC * H * D], BF16,
                                            name=f"praw_{rname}_{l}_{t0}", tag="praw")
                            for ti in range(TPC):
                                t = t0 + ti
                                nb = BPT[rname][t]
                                bof = bo[t] - bo[t0]
                                for b in range(nb):
                                    co = (bof + b) * 128
                                    psz = pzp.tile([128, W], F32, name=f"psz_{rname}_{l}_{t}_{b}", tag="pz")
                                    nc.tensor.matmul(psz[:], xs[:, xoff + co:xoff + co + 128],
                                                     cwr["wl" if kind == "gat" else "wb"][:, l * W:(l + 1) * W],
                                                     start=True, stop=False)
                                    nc.tensor.matmul(psz[:], ohS[:, co:co + 128],
                                                     xdw[:, ti * W:(ti + 1) * W],
                                                     start=False, stop=True)
                                    cp_engines[0](praw[:, (bof + b) * W:(bof + b + 1) * W], psz[:])
                                    cpi += 1

                            if kind == "gat":
                                # ---- scores: z = prelu(praw); sc = reduce(z*att)
                                sc = wrk.tile([128, NBC * H], F32, name=f"sc_{rname}_{l}_{t0}", tag="sc")
                                for ti in range(TPC):
                                    t = t0 + ti
                                    nb = BPT[rname][t]
                                    bof = bo[t] - bo[t0]
                                    zt = zp.tile([128, 5 * W], BF16, name=f"z_{rname}_{l}_{t}", tag="scrA")
                                    zv = zt[:, :nb * W]
                                    pv = praw[:, bof * W:(bof + nb) * W]
                                    nc.scalar.activation(zv, pv, AF.Prelu, alpha=0.2)
                                    attv = cwr["att"][:, l * W:(l + 1) * W]
                                    zv3 = zv.rearrange("p (b f) -> p b f", f=W)
                                    nc.vector.tensor_tensor(
                                        zv3, zv3, attv.unsqueeze(1).broadcast_to([128, nb, W]),
                                        op=OP.mult)
                                    nc.vector.tensor_reduce(
                                        sc[:, bof * H:(bof + nb) * H],
                                        zv.rearrange("p (bh f) -> p bh f", f=D),
                                        axis=mybir.AxisListType.X, op=OP.add)
                                es = wrk.tile([128, NBC * H], BF16, name=f"es_{rname}_{l}_{t0}", tag="es")
                                nc.scalar.activation(es[:, :nblk * H], sc[:, :nblk * H], AF.Exp)
                                # ---- alpha-weighted psz + aggregation
                                for ti in range(TPC):
                                    t = t0 + ti
                                    nb = BPT[rname][t]
                                    bof = bo[t] - bo[t0]
                                    xv = praw[:, bof * W:(bof + nb) * W].rearrange(
                                        "p (bh f) -> p bh f", f=D)
                                    nc.vector.tensor_tensor(
                                        xv, xv,
                                        es[:, bof * H:(bof + nb) * H].unsqueeze(2)
                                        .broadcast_to([128, nb * H, D]),
                                        op=OP.mult)
                                    pagg = paggp.tile([128, W], F32, name=f"pagg_{rname}_{l}_{t}", tag="pagg")
                                    psum_s = pssp.tile([128, H], F32, name=f"psums_{rname}_{l}_{t}", tag="pss")
                                    for b in range(nb):
                                        co = (bof + b) * 128
                                        first, last = (b == 0), (b == nb - 1)
                                        nc.tensor.matmul(pagg[:], ohA[:, co:co + 128],
                                                         praw[:, (bof + b) * W:(bof + b + 1) * W],
                                                         start=first, stop=last)
                                        nc.tensor.matmul(psum_s[:], ohA[:, co:co + 128],
                                                         es[:, (bof + b) * H:(bof + b + 1) * H],
                                                         start=first, stop=last)
                                    # ---- tile epilogue:
                                    # out_h = (pagg_h - xrW_h*s_h)/(4(s_h+eps)); sum_h; +bias
                                    asl = ACC[dty][:, t * D:(t + 1) * D]
                                    inv4 = wrk.tile([128, H], F32, name=f"inv4_{rname}_{l}_{t}", tag="inv4")
                                    nc.vector.tensor_scalar(inv4[:], psum_s[:], 1e-16, 4.0,
                                                            op0=OP.add, op1=OP.mult)
                                    nc.vector.reciprocal(inv4[:], inv4[:])
                                    nsw = wrk.tile([128, H], F32, name=f"nsw_{rname}_{l}_{t}", tag="nsw")
                                    nc.vector.tensor_tensor(nsw[:], psum_s[:], inv4[:], op=OP.mult)
                                    nc.vector.tensor_scalar(nsw[:], nsw[:], -1.0, None, op0=OP.mult)
                                    # xdw_tile *= -s*inv4 (per head), gw = pagg*inv4;
                                    # head-sum via 2-step tree; + bias
                                    xv = xdw[:, ti * W:(ti + 1) * W].rearrange("p (h f) -> p h f", f=D)
                                    nc.vector.tensor_tensor(
                                        xv, xv, nsw[:].unsqueeze(2).broadcast_to([128, H, D]),
                                        op=OP.mult)
                                    gw = wrk.tile([128, W], BF16, name=f"gw_{rname}_{l}_{t}", tag="gw")
                                    nc.vector.tensor_tensor(
                                        gw[:].rearrange("p (h f) -> p h f", f=D),
                                        pagg[:].rearrange("p (h f) -> p h f", f=D),
                                        inv4[:].unsqueeze(2).broadcast_to([128, H, D]),
                                        op=OP.mult)
                                    nc.vector.tensor_tensor(gw[:], gw[:], xdw[:, ti * W:(ti + 1) * W], op=OP.add)
                                    nc.vector.tensor_tensor(gw[:, 0:2 * D], gw[:, 0:2 * D], gw[:, 2 * D:4 * D], op=OP.add)
                                    nc.vector.tensor_tensor(gw[:, 0:D], gw[:, 0:D], gw[:, D:2 * D], op=OP.add)
                                    nc.vector.tensor_tensor(gw[:, 0:D], gw[:, 0:D],
                                                            cwr["gb"][:, l * D:(l + 1) * D], op=OP.add)
                                    if (dty, t) in acc_written:
                                        nc.vector.tensor_tensor(asl, asl, gw[:, 0:D], op=OP.add)
                                    else:
                                        nc.vector.tensor_copy(asl, gw[:, 0:D])
                                    acc_written.add((dty, t))
                            else:
                                # ---- batched CG gate/softplus chain
                                mi = praw[:, :nblk * W].rearrange("p (n c) -> p n c", c=W)
                                gatev = mi[:, :, 0:D]
                                softv = mi[:, :, D:2 * D]
                                sgA = zp.tile([128, NBC * D], F32, name=f"sgA_{rname}_{l}_{t0}", tag="scrA")
                                m_out = cgs.tile([128, NBC * D], BF16, name=f"mout_{rname}_{l}_{t0}", tag="mout")
                                Av = sgA[:, :nblk * D].rearrange("p (n c) -> p n c", c=D)
                                nc.scalar.activation(Av, gatev, AF.Exp, scale=-1.0)
                                nc.scalar.activation(
                                    m_out[:, :nblk * D].rearrange("p (n c) -> p n c", c=D),
                                    softv, AF.Exp)
                                nc.scalar.activation(m_out[:, :nblk * D], m_out[:, :nblk * D],
                                                     AF.Ln, bias=1.0)
                                nc.vector.tensor_scalar(sgA[:, :nblk * D], sgA[:, :nblk * D],
                                                        1.0, None, op0=OP.add)
                                nc.vector.reciprocal_approx_fast(sgA[:, :nblk * D], sgA[:, :nblk * D])
                                nc.vector.tensor_tensor(m_out[:, :nblk * D], m_out[:, :nblk * D],
                                                        sgA[:, :nblk * D], op=OP.mult)
                                # ---- aggregation + residual
                                for ti in range(TPC):
                                    t = t0 + ti
                                    nb = BPT[rname][t]
                                    bof = bo[t] - bo[t0]
                                    pagg = paggp.tile([128, H * D], F32, name=f"paggc_{rname}_{l}_{t}", tag="pagg")
                                    for b in range(nb):
                                        co = (bof + b) * 128
                                        nc.tensor.matmul(pagg[:, 0:D], ohA[:, co:co + 128],
                                                         m_out[:, (bof + b) * D:(bof + b + 1) * D],
                                                         start=(b == 0), stop=(b == nb - 1))
                                    asl = ACC[dty][:, t * D:(t + 1) * D]
                                    if (dty, t) in acc_written:
                                        nc.vector.tensor_tensor(asl, asl, pagg[:, 0:D], op=OP.add)
                                        nc.vector.tensor_tensor(
                                            asl, asl, xres[dty][:, t * D:(t + 1) * D], op=OP.add)
                                    else:
                                        nc.vector.scalar_tensor_tensor(
                                            asl, pagg[:, 0:D], 1.0, xres[dty][:, t * D:(t + 1) * D],
                                            op0=OP.mult, op1=OP.add)
                                    acc_written.add((dty, t))

                # ---------------- layer epilogue: nodewise linear + layout
                last_layer = (l == k_layers - 1)
                ag_in, ag_out = {}, {}
                if not last_layer:
                    for ty in ("my", "opp"):
                        ag_in[ty] = drm.tile([128, TILES * D], BF16,
                                             name=f"agin_{ty}_{l}", tag=f"agin_{ty}")
                        ag_out[ty] = drm.tile([CORES * 128, TILES * D], BF16,
                                              name=f"agout_{ty}_{l}", tag=f"agout_{ty}",
                                              addr_space="Shared")
                for tyi, ty in enumerate(("my", "opp")):
                    if ty not in {r[3] for r in rels_active}:
                        continue
                    for k in range(TILES // TPG):   # 512-col groups
                        accT = epi.tile([128, TPG * D], BF16, name=f"accT_{ty}_{l}_{k}", tag="accT")
                        for j in range(TPG):
                            t = k * TPG + j
                            ptr = psp.tile([128, 128], BF16, name=f"ptr_{ty}_{l}_{t}", tag="ps")
                            nc.tensor.transpose(ptr[:], ACC[ty][:, t * D:(t + 1) * D], ident_b[:])
                            if j % 2 == 0:
                                nc.scalar.copy(accT[:, j * D:(j + 1) * D], ptr[:])
                            else:
                                nc.vector.tensor_copy(accT[:, j * D:(j + 1) * D], ptr[:])
                        pnw = paggp.tile([128, TPG * D], F32, name=f"pnw_{ty}_{l}_{k}", tag="pagg")
                        nc.tensor.matmul(pnw[:], nw_w[:, l * D:(l + 1) * D], accT[:],
                                         start=True, stop=True)
                        if not last_layer:
                            xnk = xfm[ty][:, k * TPG * D:(k + 1) * TPG * D]
                            nc.scalar.activation(xnk, pnw[:], AF.Identity, bias=nw_b[:, l:l + 1])
                            for j in range(TPG):
                                t = k * TPG + j
                                ptr2 = psp.tile([128, 128], BF16, name=f"ptr2_{ty}_{l}_{t}", tag="ps")
                                nc.tensor.transpose(ptr2[:], xfm[ty][:, t * D:(t + 1) * D], ident_b[:])
                                nc.vector.tensor_copy(xres[ty][:, t * D:(t + 1) * D], ptr2[:])
                        else:
                            xnk = epi.tile([128, TPG * D], F32, name=f"xnT_{ty}_{l}_{k}", tag="xnT")
                            nc.scalar.activation(xnk[:], pnw[:], AF.Identity, bias=nw_b[:, l:l + 1])
                            for j in range(TPG):
                                t = k * TPG + j
                                ptr2 = psp.tile([128, 128], F32, name=f"ptr2_{ty}_{l}_{t}", tag="ps")
                                nc.tensor.transpose(ptr2[:], xnk[:, j * D:(j + 1) * D], ident_f[:])
                                osb = epi.tile([128, 128], F32, name=f"osb_{ty}_{l}_{t}", tag="osb")
                                nc.vector.tensor_copy(osb[:], ptr2[:])
                                nc.sync.dma_start(dr[f"out_{ty}"][t * 128:(t + 1) * 128, :], osb[:])
                    if not last_layer:
                        nc.sync.dma_start(ag_in[ty][:], xres[ty][:])
                        nc.gpsimd.collective_compute(
                            "AllGather", mybir.AluOpType.bypass,
                            replica_groups=[list(range(CORES))],
                            ins=[ag_in[ty].opt()], outs=[ag_out[ty].opt()],
                        )
                        nc.sync.dma_start(
                            xw[ty][:].rearrange("p (c j) -> p c j", c=CORES),
                            ag_out[ty][:].rearrange("(c p) j -> p c j", p=128),
                        )

    nc.compile()
    return nc


_prog_cache = {}


def _get_program(cfg):
    if cfg not in _prog_cache:
        _prog_cache[cfg] = _build_program(cfg)
    return _prog_cache[cfg]


# ------------------------------------------------------------------- kernel

def kernel(**inputs):
    global LAST_EXEC_NS
    from concourse.bass_utils import run_bass_kernel_spmd

    f32 = lambda k: np.asarray(inputs[k], np.float32)
    xpad = {}
    for ty, key in (("my", "x_my"), ("opp", "x_opp")):
        xp = np.zeros((NPAD, D), np.float32)
        xp[:N] = f32(key)
        xpad[ty] = xp

    BPT, packed, pos_g, order_ids = _prep_graph(inputs)
    cfg = tuple(tuple(int(v) for v in BPT[r[0]]) for r in RELS)
    nc = _get_program(cfg)

    # shared (per-core identical) tensors
    shared = {}
    for rname, kind, _, _ in RELS:
        tag = {"loses": "cg_lose", "beats": "gat_beats",
               "rev_beats": "cg_rev", "rev_loses": "gat_rev"}[rname]
        if kind == "gat":
            shared[f"wl_{rname}"] = np.ascontiguousarray(f32(f"{tag}_Wl")).astype(BF)
            shared[f"wr_{rname}"] = np.ascontiguousarray(f32(f"{tag}_Wr")).astype(BF)
            att = f32(f"{tag}_att")
            shared[f"att_{rname}"] = np.stack(
                [_rep(att[l].reshape(-1)) for l in range(L)]).astype(BF)
            b = f32(f"{tag}_b")
            shared[f"gb_{rname}"] = np.stack([_rep(b[l]) for l in range(L)])
        else:
            wf, ws = f32(f"{tag}_Wf"), f32(f"{tag}_Ws")
            shared[f"wt_{rname}"] = np.ascontiguousarray(
                np.concatenate([wf[:, :D, :], ws[:, :D, :]], axis=2)).astype(BF)
            shared[f"wb_{rname}"] = np.ascontiguousarray(
                np.concatenate([wf[:, D:, :], ws[:, D:, :]], axis=2)).astype(BF)
            bfv, bsv = f32(f"{tag}_bf"), f32(f"{tag}_bs")
            shared[f"cb_{rname}"] = np.ascontiguousarray(
                np.concatenate([bfv, bsv], axis=1).reshape(L, 1, 2 * D)).astype(BF)
    shared["nw_w"] = np.ascontiguousarray(f32("nw_W")).astype(BF)
    shared["nw_b"] = np.ascontiguousarray(f32("nw_b").reshape(L, 128, 1))
    shared["ident_f"] = np.eye(128, dtype=np.float32)
    shared["ident_b"] = np.eye(128).astype(BF)
    for ty in ("my", "opp"):
        inv = np.empty(NPAD, np.int64)
        inv[pos_g[ty]] = np.arange(NPAD)
        shared[f"xw_{ty}"] = _wrap_nodes(xpad[ty][inv])

    in_maps = []
    for c in range(CORES):
        m = dict(shared)
        for ty in ("my", "opp"):
            loc = xpad[ty][c * SHARD:(c + 1) * SHARD][order_ids[(ty, c)]]
            m[f"xres_{ty}"] = np.ascontiguousarray(
                loc.reshape(TILES, 128, D).transpose(1, 0, 2).reshape(128, TILES * D)
            ).astype(BF)
            m[f"xfm_{ty}"] = np.ascontiguousarray(loc.T).astype(BF)
        for rname, kind, _, _ in RELS:
            si, ohA, ohS = packed[(rname, c)]
            m[f"si_{rname}"] = si
            m[f"ohA_{rname}"] = ohA
            m[f"ohS_{rname}"] = ohS
        in_maps.append(m)

    trace = os.environ.get("KERNEL_PROFILE", "0") == "1"
    res = run_bass_kernel_spmd(nc, in_maps, core_ids=list(range(CORES)),
                               trace=trace, trace_cores=[0] if trace else None)
    LAST_EXEC_NS = res.exec_time_ns

    out = {}
    for ty in ("my", "opp"):
        full = np.concatenate([res.results[c][f"out_{ty}"] for c in range(CORES)])
        out[ty] = full[pos_g[ty][:N]]
    return out["my"], out["opp"]
